# revision 1
# baseline (speedup 1.0000x reference)
"""DeepseekV2 decoder layer on 8 trn2 NeuronCores.

Sharding: core c -> batch b=c//4, seq-shard j=c%4 (strided 128-token chunks
{j, 4+j, 8+j, 12+j} of the 2048-token sequence).  Each core computes the
full layer for its 512 query tokens; the compressed-kv stream (kv_a, kv_b)
is computed for all 2048 tokens on every core (replicated), so no
cross-core communication is needed.  Host code shards inputs / gathers
outputs and folds all layernorm weights + rope deinterleave permutations
into the weight matrices.
"""

import os
import sys
import functools
import numpy as np

for _p in ("/opt/trn_rl_repo", "/root/.axon_site/_ro/trn_rl_repo"):
    if os.path.isdir(_p) and _p not in sys.path:
        sys.path.insert(0, _p)
os.environ.setdefault("MYCRO_LOCAL_CACHE", "1")

B, S, H = 2, 2048, 2048
NH = 16
QLR, KVLR = 1536, 512
ROPE, NOPE, VD = 64, 128, 128
QHD = NOPE + ROPE  # 192
IM = 10944
IMP = 11264  # padded to 22*512
EPS = 1e-6
P = 128
TQ = 512  # query tokens per core
SCALE = float(QHD) ** -0.5
NEG = -1.0e9


# ---------------------------------------------------------------------------
# module builder
# ---------------------------------------------------------------------------

@functools.lru_cache(maxsize=1)
def _build():
    from contextlib import ExitStack

    import concourse.bass as bass  # noqa: F401
    from concourse import bacc, mybir, tile
    from concourse.masks import make_identity

    f32 = mybir.dt.float32
    fr = mybir.dt.float32r
    AF = mybir.ActivationFunctionType
    AX = mybir.AxisListType
    OP = mybir.AluOpType

    nc = bacc.Bacc(None, target_bir_lowering=False, debug=False)

    def di(name, shape):
        return nc.dram_tensor(name, list(shape), f32, kind="ExternalInput").ap()

    hid = di("hid", (S, H))
    xq = di("xq", (TQ, H))
    wqa = di("wqa", (H, QLR))
    wqb = di("wqb", (QLR, NH * QHD))        # reordered: nope h-major | pe deint
    wkva = di("wkva", (H, KVLR + 2 * ROPE))  # rope cols deint + duplicated
    wk = di("wk", (KVLR, NH * NOPE))
    wv = di("wv", (KVLR, NH * VD))
    wo = di("wo", (NH * VD, H))
    wg = di("wg", (H, IMP))
    wu = di("wu", (H, IMP))
    wd = di("wd", (IMP, H))
    cosq = di("cosq", (P, TQ))
    sinq = di("sinq", (P, TQ))
    cosk = di("cosk", (P, S))
    sink = di("sink", (P, S))
    cmask = di("cmask", (P, 512))
    rmat = di("rmat", (P, P))
    out = nc.dram_tensor("out", [TQ, H], f32, kind="ExternalOutput").ap()

    with tile.TileContext(nc) as tc, ExitStack() as ctx:
        def dmaf(o, i):
            nc.sync.dma_start(o.bitcast(fr), i.bitcast(fr))

        # ---------------- global pools ------------------------------
        const = ctx.enter_context(tc.tile_pool(name="const", bufs=1))
        ident = const.tile([P, P], f32, name="ident")
        make_identity(nc, ident)
        ones = const.tile([P, 1], f32, name="ones")
        nc.vector.memset(ones[:], 1.0)
        epst = const.tile([P, 1], f32, name="epst")
        nc.vector.memset(epst[:], EPS)
        cmask_s = const.tile([P, 512], f32, name="cmask_s")
        nc.sync.dma_start(cmask_s[:], cmask)
        rmat_s = const.tile([P, P], f32, name="rmat_s")
        dmaf(rmat_s[:], rmat)

        dram = ctx.enter_context(tc.tile_pool(name="dram", bufs=1,
                                              space="DRAM"))
        qtd = dram.tile([16, P, TQ], f32, name="qtd")

        ppA = ctx.enter_context(tc.tile_pool(name="ppA", bufs=4, space="PSUM"))
        ppB = ctx.enter_context(tc.tile_pool(name="ppB", bufs=2, space="PSUM"))
        ppT = ctx.enter_context(tc.tile_pool(name="ppT", bufs=2, space="PSUM"))

        def pa(n=1, w=512):
            ts = [ppA.tile([P, w], f32, tag="a", name="pa") for _ in range(n)]
            return ts if n > 1 else ts[0]

        def pb(n=1, w=512):
            ts = [ppB.tile([P, w], f32, tag="b", name="pb") for _ in range(n)]
            return ts if n > 1 else ts[0]

        def pt_(w=512):
            return ppT.tile([P, w], f32, tag="t", name="pt")

        def mm(out_, lhsT, rhs, start, stop):
            nc.tensor.matmul(out_, lhsT.bitcast(fr), rhs.bitcast(fr),
                             start=start, stop=stop)

        def mmf(out_, lhsT, rhs, start, stop):
            nc.tensor.matmul(out_, lhsT, rhs, start=start, stop=stop)

        def rms_scale(pool, dst_scale, src, n, tagp):
            """dst_scale[P,1] = 1/sqrt(mean(src^2, free) + eps) (per row)."""
            sq = pool.tile(list(src.shape), f32, tag=tagp + "sq",
                           name=tagp + "sq")
            m1 = pool.tile([src.shape[0], 1], f32, tag=tagp + "m",
                           name=tagp + "m")
            nc.scalar.activation(sq[:], src, AF.Square, accum_out=m1[:])
            srt = pool.tile([src.shape[0], 1], f32, tag=tagp + "r",
                            name=tagp + "r")
            nc.scalar.activation(srt[:], m1[:], AF.Sqrt, scale=1.0 / n,
                                 bias=epst[: src.shape[0], :])
            nc.vector.reciprocal(dst_scale, srt[:])

        # staggered-lifetime pools
        s_kvb = ExitStack()      # ckvt: A..B
        s_qtp = ExitStack()      # QTpe (right side): A..C1
        s_att = ExitStack()      # attnT (right side): B..C1
        s_c = ExitStack()        # acc: C1..C4
        p_kvb = s_kvb.enter_context(tc.tile_pool(name="p_kvb", bufs=1))
        ckvt = p_kvb.tile([P, 5, S], f32, name="ckvt")
        p_qtp = s_qtp.enter_context(
            tc.tile_pool(name="p_qtp", bufs=1, side="right"))
        QTpe = p_qtp.tile([64, 16, TQ], f32, name="QTpe")

        if True:

            def tr128(dst, src, rnd=True):
                ps = pt_()
                npart = src.shape[-1]
                nc.tensor.transpose(ps[:npart, :128], src, ident[:])
                if rnd:
                    dst = dst.bitcast(fr)
                nc.scalar.copy(dst, ps[:npart, :128])

            def tr4(dst, srcs, rnd=True):
                """Transpose up to 4 [128, w<=128] tiles into one psum bank,
                evict with a single copy.  dst free size == sum of widths."""
                ps = pt_()
                npart = srcs[0].shape[-1]
                for k, s in enumerate(srcs):
                    nc.tensor.matmul(ps[:npart, k * P:(k + 1) * P],
                                     s, ident[:], is_transpose=True,
                                     skip_group_check=True)
                if rnd:
                    dst = dst.bitcast(fr)
                nc.any.tensor_copy(dst, ps[:npart, :len(srcs) * P])

            # ========================================================
            # A1: xq -> xqt (emitted first so it overlaps the KV phase)
            # ========================================================
            s_xq = ExitStack()
            p_xq = s_xq.enter_context(tc.tile_pool(name="p_xq", bufs=1))
            xqt = p_xq.tile([P, 16, TQ], f32, name="xqt")
            sqq = [p_xq.tile([P, 1], f32, tag="sqq%d" % t,
                             name="sqq%d" % t) for t in range(4)]
            with tc.tile_pool(name="p_xs", bufs=2) as p_xs:
                for t in range(4):
                    m1 = p_xs.tile([P, 1], f32, tag="m1q", name="m1q")
                    for hf in range(4):
                        nat = p_xs.tile([P, 512], f32, tag="natq",
                                        name="natq")
                        nc.sync.dma_start(
                            nat[:], xq[t * P:(t + 1) * P,
                                       hf * 512:(hf + 1) * 512])
                        sq = p_xs.tile([P, 512], f32, tag="sqxq",
                                       name="sqxq")
                        mp = p_xs.tile([P, 1], f32, tag="mpq", name="mpq")
                        nc.scalar.activation(sq[:], nat[:], AF.Square,
                                             accum_out=mp[:])
                        if hf == 0:
                            nc.vector.tensor_copy(m1[:], mp[:])
                        else:
                            nc.vector.tensor_add(m1[:], m1[:], mp[:])
                        tr4(xqt[:, hf * 4:(hf + 1) * 4, t * P:(t + 1) * P],
                            [nat[:, k * P:(k + 1) * P] for k in range(4)])
                    srt = p_xs.tile([P, 1], f32, tag="srtq", name="srtq")
                    nc.scalar.activation(srt[:], m1[:], AF.Sqrt,
                                         scale=1.0 / H, bias=epst[:])
                    nc.vector.reciprocal(sqq[t][:], srt[:])

            # ========================================================
            # KV: hid -> X^T (quarters) -> ckv^T; token-rms; kvlr-rms;
            #     rope k_pe
            # ========================================================
            with tc.tile_pool(name="p_kv1", bufs=1) as p_kv1, \
                 tc.tile_pool(name="p_kvs", bufs=2) as p_kvs, \
                 tc.tile_pool(name="p_sr", bufs=1) as p_sr:
                sr = p_sr.tile([1, S], f32, name="sr")
                sr2 = p_sr.tile([1, S], f32, name="sr2")
                sbc = p_sr.tile([P, S], f32, name="sbc")
                msum = p_sr.tile([P, 16], f32, name="msum")
                for hf in range(4):
                    xt = p_kv1.tile([P, 4, S], f32, tag="xt", name="xt")
                    wkv = p_kv1.tile([P, 4, KVLR + 2 * ROPE], f32,
                                     tag="wkv", name="wkv")
                    for i in range(4):
                        dmaf(
                            wkv[:, i, :],
                            wkva[(hf * 4 + i) * P:(hf * 4 + i + 1) * P, :])
                    for t in range(16):
                        nat = p_kvs.tile([P, 512], f32, tag="nath",
                                         name="nath", bufs=4)
                        nc.sync.dma_start(
                            nat[:], hid[t * P:(t + 1) * P,
                                        hf * 512:(hf + 1) * 512])
                        sq = p_kvs.tile([P, 512], f32, tag="sqh", name="sqh")
                        m1 = p_kvs.tile([P, 1], f32, tag="m1h", name="m1h")
                        nc.scalar.activation(sq[:], nat[:], AF.Square,
                                             accum_out=m1[:])
                        if hf == 0:
                            nc.vector.tensor_copy(msum[:, t:t + 1], m1[:])
                        else:
                            nc.vector.tensor_add(msum[:, t:t + 1],
                                                 msum[:, t:t + 1], m1[:])
                        tr4(xt[:, :, t * P:(t + 1) * P],
                            [nat[:, k * P:(k + 1) * P] for k in range(4)])
                    for cc in range(5):
                        pk = pa(4)
                        for hcl in range(4):
                            for g in range(4):
                                mm(pk[g], wkv[:, hcl, cc * P:(cc + 1) * P],
                                   xt[:, hcl, g * 512:(g + 1) * 512],
                                   hcl == 0, hcl == 3)
                        for g in range(4):
                            dst = ckvt[:, cc, g * 512:(g + 1) * 512]
                            if hf == 0:
                                nc.scalar.copy(dst.bitcast(fr), pk[g])
                            else:
                                nc.vector.tensor_add(dst.bitcast(fr), dst,
                                                     pk[g])
                # token scale row: msum [128,16] -> [1, 2048]
                pst = pt_()
                nc.tensor.transpose(pst[0:16, :128], msum[:], ident[:])
                t16 = p_kvs.tile([16, P], f32, tag="t16", name="t16")
                nc.scalar.copy(t16[:], pst[0:16, :128])
                nc.sync.dma_start(sr[0:1, :], t16[:])
                nc.scalar.activation(sr2[:], sr[:], AF.Sqrt, scale=1.0 / H,
                                     bias=epst[0:1, :])
                nc.vector.reciprocal(sr[:], sr2[:])
                nc.gpsimd.partition_broadcast(sbc[:], sr[0:1, :])
                for cc in range(5):
                    nc.vector.tensor_mul(ckvt[:, cc, :].bitcast(fr),
                                         ckvt[:, cc, :], sbc[:])
                # kvlr rms (partition reduce via ones-matmul, fp32)
                psd = pa(4)
                for cc in range(4):
                    for g in range(4):
                        sq = p_kvs.tile([P, 512], f32, tag="sqckv",
                                        name="sqckv")
                        nc.scalar.activation(
                            sq[:], ckvt[:, cc, g * 512:(g + 1) * 512],
                            AF.Square)
                        mmf(psd[g][0:1, :], ones[:], sq[:], cc == 0, cc == 3)
                for g in range(4):
                    nc.scalar.copy(sr[0:1, g * 512:(g + 1) * 512],
                                   psd[g][0:1, :])
                nc.scalar.activation(sr2[:], sr[:], AF.Sqrt,
                                     scale=1.0 / KVLR, bias=epst[0:1, :])
                nc.vector.reciprocal(sr[:], sr2[:])
                nc.gpsimd.partition_broadcast(sbc[:], sr[0:1, :])
                for cc in range(4):
                    nc.vector.tensor_mul(ckvt[:, cc, :].bitcast(fr),
                                         ckvt[:, cc, :], sbc[:])
                # rope k_pe (chunk 4, duplicated halves)
                for g in range(4):
                    kp = ckvt[:, 4, g * 512:(g + 1) * 512]
                    ck = p_kvs.tile([P, 512], f32, tag="ckg", name="ckg")
                    sk = p_kvs.tile([P, 512], f32, tag="skg", name="skg")
                    nc.sync.dma_start(ck[:], cosk[:, g * 512:(g + 1) * 512])
                    nc.sync.dma_start(sk[:], sink[:, g * 512:(g + 1) * 512])
                    psw = pb()
                    mm(psw[:], rmat_s[:], kp, True, True)
                    t1 = p_kvs.tile([P, 512], f32, tag="krt1", name="krt1")
                    t2 = p_kvs.tile([P, 512], f32, tag="krt2", name="krt2")
                    nc.vector.tensor_mul(t1[:], kp, ck[:])
                    nc.vector.tensor_mul(t2[:], psw[:], sk[:])
                    nc.vector.tensor_add(kp.bitcast(fr), t1[:], t2[:])

            # ========================================================
            # Q path: xq -> xqt; q_a; rms; q_b -> qtd (nope) + QTpe
            # ========================================================
            with tc.tile_pool(name="p_q", bufs=1) as p_q, \
                 tc.tile_pool(name="p_qs", bufs=2) as p_qs:
                qanT = p_q.tile([P, 12, TQ], f32, name="qanT")
                cq = p_q.tile([P, TQ], f32, name="cq")
                sq_ = p_q.tile([P, TQ], f32, name="sq_")
                nc.sync.dma_start(cq[:], cosq)
                nc.sync.dma_start(sq_[:], sinq)
                with tc.tile_pool(name="p_qa", bufs=2) as p_qa:
                    # q_a (N-out, wqa read once) + rms -> qanT
                    qa_t = [p_qa.tile([P, QLR], f32, tag="qanat%d" % t,
                                      name="qanat%d" % t, bufs=1)
                            for t in range(4)]
                    for f in range(3):
                        psq = pa(4)
                        for hc in range(16):
                            w = p_qa.tile([P, 512], f32, tag="wqat",
                                          name="wqat", bufs=4)
                            dmaf(w[:], wqa[hc * P:(hc + 1) * P,
                                           f * 512:(f + 1) * 512])
                            for t in range(4):
                                mm(psq[t], xqt[:, hc, t * P:(t + 1) * P],
                                   w[:], hc == 0, hc == 15)
                        for t in range(4):
                            nc.vector.tensor_scalar_mul(
                                qa_t[t][:, f * 512:(f + 1) * 512], psq[t],
                                sqq[t][:])
                    for t in range(4):
                        qa = qa_t[t]
                        s2 = p_qa.tile([P, 1], f32, tag="s2", name="s2")
                        rms_scale(p_qa, s2[:], qa[:], QLR, "qa")
                        nc.vector.tensor_scalar_mul(qa[:], qa[:], s2[:])
                        for g in range(3):
                            tr4(qanT[:, 4 * g:4 * (g + 1),
                                     t * P:(t + 1) * P],
                                [qa[:, (4 * g + k) * P:(4 * g + k + 1) * P]
                                 for k in range(4)])
                # q_b (N-out, wqb read once): nope -> qtd, pe -> QTpe
                with tc.tile_pool(name="p_qb", bufs=2) as p_qb:
                    qn_t = [p_qb.tile([P, NH * QHD], f32, tag="qn%d" % t,
                                      name="qn%d" % t, bufs=1)
                            for t in range(4)]
                    for f in range(6):
                        psb = pa(4)
                        for lc in range(12):
                            w = p_qb.tile([P, 512], f32, tag="wqbt",
                                          name="wqbt", bufs=4)
                            dmaf(w[:], wqb[lc * P:(lc + 1) * P,
                                           f * 512:(f + 1) * 512])
                            for t in range(4):
                                mm(psb[t], qanT[:, lc, t * P:(t + 1) * P],
                                   w[:], lc == 0, lc == 11)
                        for t in range(4):
                            nc.any.tensor_copy(
                                qn_t[t][:, f * 512:(f + 1) * 512], psb[t])
                    for t in range(4):
                        qn = qn_t[t]
                        qtr = qtd.rearrange("c p f -> p c f")
                        for g in range(4):
                            st = p_qb.tile([P, 512], f32, tag="qstage",
                                           name="qstage")
                            tr4(st[:],
                                [qn[:, (4 * g + k) * P:(4 * g + k + 1) * P]
                                 for k in range(4)], rnd=False)
                            nc.sync.dma_start(
                                qtr[:, 4 * g:4 * (g + 1),
                                    t * P:(t + 1) * P], st[:])
                        for g in range(4):
                            tr4(QTpe[:, 4 * g:4 * (g + 1),
                                     t * P:(t + 1) * P],
                                [qn[:, 2048 + 64 * (4 * g + k):
                                     2048 + 64 * (4 * g + k + 1)]
                                 for k in range(4)])
                # rope q_pe (per head, partition base 0)
                for h in range(NH):
                    qd = QTpe[:, h, :]
                    psw = pb()
                    mm(psw[0:64, :], rmat_s[0:64, 0:64], qd, True, True)
                    t1 = p_qs.tile([P, TQ], f32, tag="qrt1", name="qrt1")
                    t2 = p_qs.tile([P, TQ], f32, tag="qrt2", name="qrt2")
                    nc.vector.tensor_mul(t1[0:64, :], qd, cq[0:64, :])
                    nc.vector.tensor_mul(t2[0:64, :], psw[0:64, :],
                                         sq_[0:64, :])
                    nc.vector.tensor_add(qd.bitcast(fr), t1[0:64, :],
                                         t2[0:64, :])

            s_xq.close()

            # ========================================================
            # Attention per head
            # ========================================================
            p_at = s_att.enter_context(tc.tile_pool(name="p_at", bufs=1, side="right"))
            attnT = p_at.tile([P, 16, TQ], f32, name="attnT")
            with tc.tile_pool(name="p_b1", bufs=1) as p_b1, \
                 tc.tile_pool(name="p_bs", bufs=2) as p_bs, \
                 tc.tile_pool(name="p_bs2", bufs=2) as _unused_bs2:
                for h in range(NH):
                    hb = 64 * (h % 2)
                    chq = h // 2
                    kt = p_b1.tile([P, S], f32, tag="kt", name="kt", bufs=2)
                    vt = p_b1.tile([P, 16, VD], f32, tag="vt", name="vt", bufs=2)
                    qh = p_bs.tile([P, TQ], f32, tag="qh", name="qh")
                    dmaf(qh[:], qtd[h])
                    wkh = p_bs.tile([P, 4, NOPE], f32, tag="wkh", name="wkh")
                    wvh = p_bs.tile([P, 4, VD], f32, tag="wvh", name="wvh")
                    wkr = wk.rearrange("(c p) f -> p c f", p=P)
                    wvr = wv.rearrange("(c p) f -> p c f", p=P)
                    dmaf(wkh[:], wkr[:, :, h * NOPE:(h + 1) * NOPE])
                    dmaf(wvh[:], wvr[:, :, h * VD:(h + 1) * VD])
                    pk = pa(4)
                    for kc in range(4):
                        for g in range(4):
                            mm(pk[g], wkh[:, kc, :],
                               ckvt[:, kc, g * 512:(g + 1) * 512],
                               kc == 0, kc == 3)
                    for g in range(4):
                        nc.any.tensor_copy(
                            kt[:, g * 512:(g + 1) * 512].bitcast(fr), pk[g])
                    pv = pa(4)
                    for kc in range(4):
                        for g in range(4):
                            mm(pv[g], wvh[:, kc, :],
                               ckvt[:, kc, g * 512:(g + 1) * 512],
                               kc == 0, kc == 3)
                    for g in range(4):
                        vtm = p_bs.tile([P, 512], f32, tag="vtm", name="vtm")
                        nc.any.tensor_copy(vtm[:], pv[g])
                        tr4(vt[:, g * 4:g * 4 + 4, :],
                            [vtm[:, k * P:(k + 1) * P] for k in range(4)])
                    for i in range(4):
                        qsl = slice(i * P, (i + 1) * P)
                        pn = p_b1.tile([P, 4, 512], f32, tag="pn", name="pn",
                                       bufs=2)
                        dn = p_bs.tile([P, 4], f32, tag="dn", name="dn")
                        mx = p_bs.tile([P, 4], f32, tag="mx", name="mx")
                        for kg in range(i + 1):
                            ps = pb()
                            mm(ps, qh[:, qsl],
                               kt[:, kg * 512:(kg + 1) * 512], True, False)
                            mm(ps, QTpe[:, h, qsl],
                               ckvt[0:64, 4, kg * 512:(kg + 1) * 512],
                               False, True)
                            if kg == i:
                                nc.vector.tensor_add(ps, ps, cmask_s[:])
                            nc.vector.tensor_reduce(
                                mx[:, kg:kg + 1], ps, AX.X, OP.max)
                            nc.any.tensor_copy(pn[:, kg, :].bitcast(fr), ps)
                        gmx = p_bs.tile([P, 1], f32, tag="gmx", name="gmx")
                        nc.vector.tensor_reduce(gmx[:], mx[:, 0:i + 1],
                                                AX.X, OP.max)
                        nb = p_bs.tile([P, 1], f32, tag="nb", name="nb")
                        nc.vector.tensor_scalar_mul(nb[:], gmx[:], -SCALE)
                        for kg in range(i + 1):
                            nc.scalar.activation(
                                pn[:, kg, :].bitcast(fr), pn[:, kg, :],
                                AF.Exp, scale=SCALE, bias=nb[:],
                                accum_out=dn[:, kg:kg + 1])
                        ds = p_bs.tile([P, 1], f32, tag="ds", name="ds")
                        nc.vector.tensor_reduce(ds[:], dn[:, 0:i + 1],
                                                AX.X, OP.add)
                        dr = p_bs.tile([P, 1], f32, tag="dr", name="dr")
                        nc.vector.reciprocal(dr[:], ds[:])
                        # diag(1/denom): fused normalize inside transpose-mm
                        dt_ = p_bs.tile([P, P], f32, tag="dt_", name="dt_")
                        nc.vector.tensor_scalar_mul(dt_[:].bitcast(fr),
                                                    ident[:], dr[:])
                        PTs = p_b1.tile([P, 16, P], f32, tag="PTs",
                                        name="PTs", bufs=2)
                        for kg in range(i + 1):
                            ps2 = pt_()
                            for k in range(4):
                                nc.tensor.matmul(
                                    ps2[:, k * P:(k + 1) * P],
                                    pn[:, kg, k * P:(k + 1) * P].bitcast(fr),
                                    dt_[:].bitcast(fr), start=True, stop=True,
                                    skip_group_check=True)
                            nc.any.tensor_copy(
                                PTs[:, 4 * kg:4 * (kg + 1), :].bitcast(fr),
                                ps2[:, 0:512])
                        # per-slot AV
                        pav = pa()
                        nkc = 4 * (i + 1)
                        for kc in range(nkc):
                            mm(pav[:, 0:P], vt[:, kc, :], PTs[:, kc, :],
                               kc == 0, kc == nkc - 1)
                        nc.any.tensor_copy(
                            attnT[:, h, qsl].bitcast(fr), pav[:, 0:P])
            s_kvb.close()

        # ============================================================
        # C: o-proj + residual; MLP
        # ============================================================
        p_c = s_c.enter_context(tc.tile_pool(name="p_c", bufs=1))
        acc = [p_c.tile([P, H], f32, tag="acc%d" % t, name="acc%d" % t)
               for t in range(4)]
        with tc.tile_pool(name="p_cs", bufs=2) as p_cs:

            def tr128c(dst, src):
                ps = pt_()
                nc.tensor.transpose(ps[:, :128], src, ident[:])
                nc.scalar.copy(dst.bitcast(fr), ps[:, :128])

            with tc.tile_pool(name="p_co", bufs=2) as p_co:
                for f in range(4):
                    pso = pa(4)
                    for hc in range(16):
                        w = p_co.tile([P, 512], f32, tag="wot", name="wot", bufs=4)
                        dmaf(w[:], wo[hc * P:(hc + 1) * P,
                                      f * 512:(f + 1) * 512])
                        for t in range(4):
                            mm(pso[t], attnT[:, hc, t * P:(t + 1) * P],
                               w[:], hc == 0, hc == 15)
                    for t in range(4):
                        res = p_co.tile([P, 512], f32, tag="res", name="res")
                        nc.sync.dma_start(
                            res[:], xq[t * P:(t + 1) * P,
                                       f * 512:(f + 1) * 512])
                        nc.vector.tensor_add(
                            acc[t][:, f * 512:(f + 1) * 512],
                            pso[t], res[:])
            s_att.close()
            s_qtp.close()

            # y = rms(h1) -> yT
            yT = p_c.tile([P, 16, TQ], f32, name="yT")
            for t in range(4):
                s3 = p_cs.tile([P, 1], f32, tag="s3", name="s3")
                rms_scale(p_cs, s3[:], acc[t][:], H, "y2ksq")
                yn = p_cs.tile([P, H], f32, tag="y2ksqsq", name="yn")
                nc.vector.tensor_scalar_mul(yn[:], acc[t][:], s3[:])
                for g in range(4):
                    tr4(yT[:, 4 * g:4 * (g + 1), t * P:(t + 1) * P],
                        [yn[:, (4 * g + k) * P:(4 * g + k + 1) * P]
                         for k in range(4)])

            # MLP
            with tc.tile_pool(name="wbig", bufs=3) as wbig, \
                 tc.tile_pool(name="mtp", bufs=2) as mtp:
                for fs in range(IMP // 512):
                    wgt = wbig.tile([P, 16, 512], f32, tag="wbig",
                                    name="wgt")
                    wut = wbig.tile([P, 16, 512], f32, tag="wbig",
                                    name="wut")
                    for hc in range(16):
                        dmaf(wgt[:, hc, :],
                                          wg[hc * P:(hc + 1) * P,
                                             fs * 512:(fs + 1) * 512])
                        dmaf(wut[:, hc, :],
                                          wu[hc * P:(hc + 1) * P,
                                             fs * 512:(fs + 1) * 512])
                    wdt = wbig.tile([P, 4, H], f32, tag="wbig", name="wdt")
                    for ic in range(4):
                        dmaf(wdt[:, ic, :],
                                          wd[(fs * 4 + ic) * P:
                                             (fs * 4 + ic + 1) * P, :])
                    mt = mtp.tile([P, 4, TQ], f32, tag="mt", name="mt", bufs=1)
                    for t in range(4):
                        psg, psu = pb(2)
                        for hc in range(16):
                            mm(psg, yT[:, hc, t * P:(t + 1) * P],
                               wgt[:, hc, :], hc == 0, hc == 15)
                            mm(psu, yT[:, hc, t * P:(t + 1) * P],
                               wut[:, hc, :], hc == 0, hc == 15)
                        gs = p_cs.tile([P, 512], f32, tag="gs", name="gs")
                        nc.scalar.activation(gs[:], psg, AF.Silu)
                        mn = p_cs.tile([P, 512], f32, tag="mn", name="mn")
                        nc.vector.tensor_mul(mn[:], gs[:], psu)
                        tr4(mt[:, :, t * P:(t + 1) * P],
                            [mn[:, k * P:(k + 1) * P] for k in range(4)])
                    for t in range(4):
                        psd = pa(4)
                        for ic in range(4):
                            for f in range(4):
                                mm(psd[f], mt[:, ic, t * P:(t + 1) * P],
                                   wdt[:, ic, f * 512:(f + 1) * 512],
                                   ic == 0, ic == 3)
                        for f in range(4):
                            nc.vector.tensor_add(
                                acc[t][:, f * 512:(f + 1) * 512],
                                acc[t][:, f * 512:(f + 1) * 512], psd[f])

            for t in range(4):
                nc.sync.dma_start(out[t * P:(t + 1) * P, :], acc[t][:])
        s_c.close()

    nc.compile()
    return nc


# ---------------------------------------------------------------------------
# host side
# ---------------------------------------------------------------------------

_DEINT = np.concatenate([np.arange(0, ROPE, 2), np.arange(1, ROPE, 2)])


def _rmat():
    r = np.zeros((P, P), np.float32)
    for m in range(P):
        base = (m // 64) * 64
        k = base + ((m - base) + 32) % 64
        r[k, m] = 1.0
    return r


def _prep_core(c, hs, ins):
    b, j = c // 4, c % 4
    qch = [j, 4 + j, 8 + j, 12 + j]
    qrows = np.concatenate([np.arange(ch * P, (ch + 1) * P) for ch in qch])

    pos = np.asarray(ins["position_ids"])[b].astype(np.int64)
    cosg = np.asarray(ins["cos"])[pos]           # [S, 64]
    sing = np.asarray(ins["sin"])[pos]
    ssgn = np.concatenate([-sing[:, :32], sing[:, 32:]], 1)

    def dup(x):  # [S,64] -> [128, S]
        xt = np.ascontiguousarray(x.T.astype(np.float32))
        return np.concatenate([xt, xt], 0)

    iln = np.asarray(ins["input_ln_w"])[:, None]
    qln = np.asarray(ins["q_a_ln_w"])[:, None]
    kln = np.asarray(ins["kv_a_ln_w"])[:, None]
    pln = np.asarray(ins["post_ln_w"])[:, None]

    wqa = (iln * np.asarray(ins["q_a_kernel"])).astype(np.float32)
    wqb_ = (qln * np.asarray(ins["q_b_kernel"])).reshape(QLR, NH, QHD)
    wqb = np.concatenate(
        [wqb_[:, :, :NOPE].reshape(QLR, NH * NOPE),
         wqb_[:, :, NOPE:][:, :, _DEINT].reshape(QLR, NH * ROPE)], 1)
    kva = iln * np.asarray(ins["kv_a_kernel"])
    rope_d = kva[:, KVLR:][:, _DEINT]
    wkva = np.concatenate([kva[:, :KVLR], rope_d, rope_d], 1)
    wkb = (kln * np.asarray(ins["kv_b_kernel"])).reshape(KVLR, NH, NOPE + VD)
    wk = wkb[:, :, :NOPE].reshape(KVLR, NH * NOPE)
    wv = wkb[:, :, NOPE:].reshape(KVLR, NH * VD)
    wg = np.zeros((H, IMP), np.float32)
    wg[:, :IM] = pln * np.asarray(ins["gate_kernel"])
    wu = np.zeros((H, IMP), np.float32)
    wu[:, :IM] = pln * np.asarray(ins["up_kernel"])
    wd = np.zeros((IMP, H), np.float32)
    wd[:IM, :] = np.asarray(ins["down_kernel"])

    cmask = np.where(
        np.arange(512)[None, :] <= P * j + np.arange(P)[:, None],
        np.float32(0), np.float32(NEG)).astype(np.float32)

    f32c = lambda x: np.ascontiguousarray(x, dtype=np.float32)
    return {
        "hid": f32c(hs[b]),
        "xq": f32c(hs[b][qrows]),
        "wqa": f32c(wqa),
        "wqb": f32c(wqb),
        "wkva": f32c(wkva),
        "wk": f32c(wk),
        "wv": f32c(wv),
        "wo": f32c(np.asarray(ins["o_kernel"])),
        "wg": f32c(wg),
        "wu": f32c(wu),
        "wd": f32c(wd),
        "cosq": f32c(dup(cosg)[:, qrows]),
        "sinq": f32c(dup(ssgn)[:, qrows]),
        "cosk": f32c(dup(cosg)),
        "sink": f32c(dup(ssgn)),
        "cmask": cmask,
        "rmat": _rmat(),
    }, qrows


def kernel(**inputs):
    from concourse import bass_utils

    hs = np.asarray(inputs["hidden_states"], dtype=np.float32)
    in_maps, qrows_l = [], []
    for c in range(8):
        m, qr = _prep_core(c, hs, inputs)
        in_maps.append(m)
        qrows_l.append(qr)

    nc = _build()
    res = bass_utils.run_bass_kernel_spmd(
        nc, in_maps, core_ids=list(range(8)))

    out = np.empty((B, S, H), np.float32)
    for c in range(8):
        out[c // 4, qrows_l[c]] = res.results[c]["out"]
    return out



# revision 7
# speedup vs baseline: 1.2248x; 1.2248x over previous
"""DeepseekV2 decoder layer on 8 trn2 NeuronCores.

Sharding: core c -> batch b=c//4, seq-shard j=c%4 (strided 128-token chunks
{j, 4+j, 8+j, 12+j} of the 2048-token sequence).  Each core computes the
full layer for its 512 query tokens; the compressed-kv stream (kv_a, kv_b)
is computed for all 2048 tokens on every core (replicated), so no
cross-core communication is needed.  Host code shards inputs / gathers
outputs and folds all layernorm weights + rope deinterleave permutations
into the weight matrices.

Attention runs in transposed-softmax layout: scores are computed as
S^T [keys, queries], exp'd without max subtraction (scores are bounded),
the denominator comes from a ones-matmul partition reduce, and AV
consumes the probabilities directly - no probability transposes.
Most operands are bf16 (fp32 PSUM accumulation everywhere).
"""

import os
import sys
import functools
import numpy as np
import ml_dtypes

for _p in ("/opt/trn_rl_repo", "/root/.axon_site/_ro/trn_rl_repo"):
    if os.path.isdir(_p) and _p not in sys.path:
        sys.path.insert(0, _p)
os.environ.setdefault("MYCRO_LOCAL_CACHE", "1")

B, S, H = 2, 2048, 2048
NH = 16
QLR, KVLR = 1536, 512
ROPE, NOPE, VD = 64, 128, 128
QHD = NOPE + ROPE  # 192
IM = 10944
IMP = 11264  # padded to 22*512
EPS = 1e-6
P = 128
TQ = 512  # query tokens per core
SCALE = float(QHD) ** -0.5
NEG = -1.0e9

# span start (in q chunks of 128) for each key chunk kc; kc 12-15 padded
# from 3 to 2 so every score matmul has free size >= 256
FQ = [0, 0, 0, 0, 1, 1, 1, 1, 2, 2, 2, 2, 2, 2, 2, 2]
# number of 128-q-chunks of mask to add for each kc (starting at FQ[kc])
MW = [1] * 12 + [2] * 4


# ---------------------------------------------------------------------------
# module builder
# ---------------------------------------------------------------------------

@functools.lru_cache(maxsize=1)
def _build():
    from contextlib import ExitStack

    import concourse.bass as bass  # noqa: F401
    from concourse import bacc, mybir, tile
    from concourse.masks import make_identity

    f32 = mybir.dt.float32
    bf16 = mybir.dt.bfloat16
    fr = mybir.dt.float32r
    AF = mybir.ActivationFunctionType
    AX = mybir.AxisListType
    OP = mybir.AluOpType

    nc = bacc.Bacc(None, target_bir_lowering=False, debug=False)

    def di(name, shape, dt=f32):
        return nc.dram_tensor(name, list(shape), dt, kind="ExternalInput").ap()

    hidb = di("hidb", (S, H), bf16)
    xqb = di("xqb", (TQ, H), bf16)
    xq = di("xq", (TQ, H))
    wqa = di("wqa", (H, QLR), bf16)
    wqb = di("wqb", (QLR, NH * QHD), bf16)     # nope h-major | pe deint
    wkva = di("wkva", (H, KVLR + 2 * ROPE), bf16)  # rope cols deint + dup
    wk = di("wk", (KVLR, NH * NOPE), bf16)
    wv = di("wv", (KVLR, NH * VD), bf16)
    wo = di("wo", (NH * VD, H), bf16)
    wg = di("wg", (H, IMP))
    wu = di("wu", (H, IMP))
    wd = di("wd", (IMP, H))
    cosq = di("cosq", (P, TQ), bf16)
    sinq = di("sinq", (P, TQ), bf16)
    cosk = di("cosk", (P, S))
    sink = di("sink", (P, S))
    maskt = di("maskt", (P, 16 * 256))
    rmat = di("rmat", (P, P))
    rmatb = di("rmatb", (P, P), bf16)
    out = nc.dram_tensor("out", [TQ, H], f32, kind="ExternalOutput").ap()

    with tile.TileContext(nc) as tc, ExitStack() as ctx:
        def dmaf(o, i):
            nc.sync.dma_start(o.bitcast(fr), i.bitcast(fr))

        # ---------------- global pools ------------------------------
        const = ctx.enter_context(tc.tile_pool(name="const", bufs=1))
        ident = const.tile([P, P], f32, name="ident")
        make_identity(nc, ident)
        identb = const.tile([P, P], bf16, name="identb")
        nc.any.tensor_copy(identb[:], ident[:])
        ones = const.tile([P, 1], f32, name="ones")
        nc.vector.memset(ones[:], 1.0)
        onesb = const.tile([P, 1], bf16, name="onesb")
        nc.vector.memset(onesb[:], 1.0)
        epst = const.tile([P, 1], f32, name="epst")
        nc.vector.memset(epst[:], EPS)
        rmat_s = const.tile([P, P], f32, name="rmat_s")
        dmaf(rmat_s[:], rmat)
        rmatb_s = const.tile([P, P], bf16, name="rmatb_s")
        nc.sync.dma_start(rmatb_s[:], rmatb)

        ppA = ctx.enter_context(tc.tile_pool(name="ppA", bufs=4, space="PSUM"))
        ppB = ctx.enter_context(tc.tile_pool(name="ppB", bufs=2, space="PSUM"))
        ppT = ctx.enter_context(tc.tile_pool(name="ppT", bufs=2, space="PSUM"))

        def pa(n=1, w=512):
            ts = [ppA.tile([P, w], f32, tag="a", name="pa") for _ in range(n)]
            return ts if n > 1 else ts[0]

        def pb(n=1, w=512):
            ts = [ppB.tile([P, w], f32, tag="b", name="pb") for _ in range(n)]
            return ts if n > 1 else ts[0]

        def pt_(w=512, dt=f32):
            return ppT.tile([P, w], dt, tag="t", name="pt")

        def mm(out_, lhsT, rhs, start, stop):
            nc.tensor.matmul(out_, lhsT.bitcast(fr), rhs.bitcast(fr),
                             start=start, stop=stop)

        def mmb(out_, lhsT, rhs, start, stop, skip=False):
            nc.tensor.matmul(out_, lhsT, rhs, start=start, stop=stop,
                             skip_group_check=skip)

        def rms_scale(pool, dst_scale, src, n, tagp):
            """dst_scale[P,1] = 1/sqrt(mean(src^2, free) + eps) (per row)."""
            sq = pool.tile(list(src.shape), f32, tag=tagp + "sq",
                           name=tagp + "sq")
            m1 = pool.tile([src.shape[0], 1], f32, tag=tagp + "m",
                           name=tagp + "m")
            nc.scalar.activation(sq[:], src, AF.Square, accum_out=m1[:])
            srt = pool.tile([src.shape[0], 1], f32, tag=tagp + "r",
                            name=tagp + "r")
            nc.scalar.activation(srt[:], m1[:], AF.Sqrt, scale=1.0 / n,
                                 bias=epst[: src.shape[0], :])
            nc.vector.reciprocal(dst_scale, srt[:])

        def tr4b(dst, srcs):
            """Transpose up to 4 [128, w<=128] bf16 tiles into one psum
            bank, evict with a single copy."""
            ps = pt_(512, bf16)
            npart = srcs[0].shape[-1]
            for k, s in enumerate(srcs):
                nc.tensor.matmul(ps[:npart, k * P:(k + 1) * P],
                                 s, identb[:], is_transpose=True,
                                 skip_group_check=True)
            nc.any.tensor_copy(dst, ps[:npart, :len(srcs) * P])

        def tr4(dst, srcs, rnd=True):
            ps = pt_()
            npart = srcs[0].shape[-1]
            for k, s in enumerate(srcs):
                nc.tensor.matmul(ps[:npart, k * P:(k + 1) * P],
                                 s, ident[:], is_transpose=True,
                                 skip_group_check=True)
            if rnd:
                dst = dst.bitcast(fr)
            nc.any.tensor_copy(dst, ps[:npart, :len(srcs) * P])

        # staggered-lifetime pools
        s_kvb = ExitStack()      # ckvt bf16: lives until end of attention
        s_qtp = ExitStack()      # qT / QTpe2: until last head's scores
        s_att = ExitStack()      # attnT: until o-proj
        s_c = ExitStack()        # acc: o-proj..end
        p_kvb = s_kvb.enter_context(tc.tile_pool(name="p_kvb", bufs=1))
        ckvt = p_kvb.tile([P, 5, S], bf16, name="ckvt")
        p_qtp = s_qtp.enter_context(
            tc.tile_pool(name="p_qtp", bufs=1, side="right"))
        qT = p_qtp.tile([P, NH, TQ], bf16, name="qT")
        QTpe2 = p_qtp.tile([P, 8, TQ], bf16, name="QTpe2")

        if True:
            # ========================================================
            # A1: xq -> xqt (bf16) + q-token rms scales
            # ========================================================
            s_xq = ExitStack()
            p_xq = s_xq.enter_context(tc.tile_pool(name="p_xq", bufs=1))
            xqt = p_xq.tile([P, 16, TQ], bf16, name="xqt")
            sqq = [p_xq.tile([P, 1], f32, tag="sqq%d" % t,
                             name="sqq%d" % t) for t in range(4)]
            with tc.tile_pool(name="p_xs", bufs=2) as p_xs:
                for t in range(4):
                    nat = p_xs.tile([P, 4, 512], bf16, tag="natq",
                                    name="natq")
                    nc.sync.dma_start(
                        nat[:], xqb[t * P:(t + 1) * P, :].rearrange(
                            "p (c f) -> p c f", f=512))
                    m1 = p_xs.tile([P, 1], f32, tag="m1q", name="m1q")
                    for hf in range(4):
                        sq = p_xs.tile([P, 512], f32, tag="sqxq",
                                       name="sqxq")
                        mp = p_xs.tile([P, 1], f32, tag="mpq", name="mpq")
                        nc.scalar.activation(sq[:], nat[:, hf, :], AF.Square,
                                             accum_out=mp[:])
                        if hf == 0:
                            nc.vector.tensor_copy(m1[:], mp[:])
                        else:
                            nc.vector.tensor_add(m1[:], m1[:], mp[:])
                        tr4b(xqt[:, hf * 4:(hf + 1) * 4, t * P:(t + 1) * P],
                             [nat[:, hf, k * P:(k + 1) * P]
                              for k in range(4)])
                    srt = p_xs.tile([P, 1], f32, tag="srtq", name="srtq")
                    nc.scalar.activation(srt[:], m1[:], AF.Sqrt,
                                         scale=1.0 / H, bias=epst[:])
                    nc.vector.reciprocal(sqq[t][:], srt[:])

            s_ckacc = ExitStack()
            p_cka = s_ckacc.enter_context(
                tc.tile_pool(name="p_cka", bufs=1))
            ckacc = p_cka.tile([P, 5, S], f32, name="ckacc")

            # ========================================================
            # KV: hid -> X^T (bf16) -> ckv^T(f32); token-rms; kvlr-rms;
            #     rope k_pe; cast ckvt -> bf16
            # ========================================================
            with tc.tile_pool(name="p_kv1", bufs=1) as p_kv1, \
                 tc.tile_pool(name="p_kvs", bufs=2) as p_kvs, \
                 tc.tile_pool(name="p_sr", bufs=1) as p_sr:
                sr = p_sr.tile([1, S], f32, name="sr")
                sr2 = p_sr.tile([1, S], f32, name="sr2")
                sbc = p_sr.tile([P, S], f32, name="sbc")
                msum = p_sr.tile([P, 16], f32, name="msum")
                hidr = hidb.rearrange("(c p) f -> p c f", p=P)
                wkvr = wkva.rearrange("(c p) f -> p c f", p=P)
                for hf in range(4):
                    xt = p_kv1.tile([P, 4, S], bf16, tag="xt", name="xt")
                    wkv = p_kv1.tile([P, 4, KVLR + 2 * ROPE], bf16,
                                     tag="wkv", name="wkv")
                    nc.sync.dma_start(wkv[:], wkvr[:, 4 * hf:4 * hf + 4, :])
                    xh = p_kv1.tile([P, 16, 512], bf16, tag="xh", name="xh")
                    nc.sync.dma_start(
                        xh[:], hidr[:, :, hf * 512:(hf + 1) * 512])
                    for t in range(16):
                        sq = p_kvs.tile([P, 512], f32, tag="sqh", name="sqh")
                        m1 = p_kvs.tile([P, 1], f32, tag="m1h", name="m1h")
                        nc.scalar.activation(sq[:], xh[:, t, :], AF.Square,
                                             accum_out=m1[:])
                        if hf == 0:
                            nc.vector.tensor_copy(msum[:, t:t + 1], m1[:])
                        else:
                            nc.vector.tensor_add(msum[:, t:t + 1],
                                                 msum[:, t:t + 1], m1[:])
                        tr4b(xt[:, :, t * P:(t + 1) * P],
                             [xh[:, t, k * P:(k + 1) * P] for k in range(4)])
                    for cc in range(5):
                        pk = pa(4)
                        for hcl in range(4):
                            for g in range(4):
                                mmb(pk[g], wkv[:, hcl, cc * P:(cc + 1) * P],
                                    xt[:, hcl, g * 512:(g + 1) * 512],
                                    hcl == 0, hcl == 3)
                        for g in range(4):
                            dst = ckacc[:, cc, g * 512:(g + 1) * 512]
                            if hf == 0:
                                nc.scalar.copy(dst.bitcast(fr), pk[g])
                            else:
                                nc.vector.tensor_add(dst.bitcast(fr), dst,
                                                     pk[g])
                # token scale row: msum [128,16] -> [1, 2048]
                pst = pt_()
                nc.tensor.transpose(pst[0:16, :128], msum[:], ident[:])
                t16 = p_kvs.tile([16, P], f32, tag="t16", name="t16")
                nc.scalar.copy(t16[:], pst[0:16, :128])
                nc.sync.dma_start(sr[0:1, :], t16[:])
                nc.scalar.activation(sr2[:], sr[:], AF.Sqrt, scale=1.0 / H,
                                     bias=epst[0:1, :])
                nc.vector.reciprocal(sr[:], sr2[:])
                nc.gpsimd.partition_broadcast(sbc[:], sr[0:1, :])
                for cc in range(5):
                    nc.vector.tensor_mul(ckacc[:, cc, :].bitcast(fr),
                                         ckacc[:, cc, :], sbc[:])
                # kvlr rms (partition reduce via ones-matmul)
                psd = pa(4)
                for cc in range(4):
                    for g in range(4):
                        sq = p_kvs.tile([P, 512], f32, tag="sqckv",
                                        name="sqckv")
                        nc.scalar.activation(
                            sq[:], ckacc[:, cc, g * 512:(g + 1) * 512],
                            AF.Square)
                        mm(psd[g][0:1, :], ones[:], sq[:], cc == 0, cc == 3)
                for g in range(4):
                    nc.scalar.copy(sr[0:1, g * 512:(g + 1) * 512],
                                   psd[g][0:1, :])
                nc.scalar.activation(sr2[:], sr[:], AF.Sqrt,
                                     scale=1.0 / KVLR, bias=epst[0:1, :])
                nc.vector.reciprocal(sr[:], sr2[:])
                nc.gpsimd.partition_broadcast(sbc[:], sr[0:1, :])
                for cc in range(4):
                    nc.vector.tensor_mul(ckacc[:, cc, :].bitcast(fr),
                                         ckacc[:, cc, :], sbc[:])
                # rope k_pe (chunk 4, duplicated halves)
                for g in range(4):
                    kp = ckacc[:, 4, g * 512:(g + 1) * 512]
                    ck = p_kvs.tile([P, 512], f32, tag="ckg", name="ckg")
                    sk = p_kvs.tile([P, 512], f32, tag="skg", name="skg")
                    nc.sync.dma_start(ck[:], cosk[:, g * 512:(g + 1) * 512])
                    nc.sync.dma_start(sk[:], sink[:, g * 512:(g + 1) * 512])
                    psw = pb()
                    mm(psw[:], rmat_s[:], kp, True, True)
                    t1 = p_kvs.tile([P, 512], f32, tag="krt1", name="krt1")
                    t2 = p_kvs.tile([P, 512], f32, tag="krt2", name="krt2")
                    nc.vector.tensor_mul(t1[:], kp, ck[:])
                    nc.vector.tensor_mul(t2[:], psw[:], sk[:])
                    nc.vector.tensor_add(kp.bitcast(fr), t1[:], t2[:])
                # cast full ckv stream to bf16
                for cc in range(5):
                    for g in range(2):
                        nc.any.tensor_copy(
                            ckvt[:, cc, g * 1024:(g + 1) * 1024],
                            ckacc[:, cc, g * 1024:(g + 1) * 1024])
            s_ckacc.close()

            # ========================================================
            # Q path: q_a (token-major, rms) -> qanT (bf16);
            # q_b^T direct -> qT (nope) + QTpe2 (rope, 2 heads/slot);
            # rope q
            # ========================================================
            with tc.tile_pool(name="p_q", bufs=1) as p_q, \
                 tc.tile_pool(name="p_qs", bufs=2) as p_qs:
                qanT = p_q.tile([P, 12, TQ], bf16, name="qanT")
                cq = p_q.tile([P, TQ], bf16, name="cq")
                sq_ = p_q.tile([P, TQ], bf16, name="sq_")
                nc.sync.dma_start(cq[:], cosq)
                nc.sync.dma_start(sq_[:], sinq)
                wqar = wqa.rearrange("(c p) f -> p c f", p=P)
                with tc.tile_pool(name="p_qa", bufs=2) as p_qa:
                    qa_t = [p_qa.tile([P, QLR], f32, tag="qanat%d" % t,
                                      name="qanat%d" % t, bufs=1)
                            for t in range(4)]
                    for f in range(3):
                        psq = pa(4)
                        w = p_qa.tile([P, 16, 512], bf16, tag="wqat",
                                      name="wqat")
                        nc.sync.dma_start(
                            w[:], wqar[:, :, f * 512:(f + 1) * 512])
                        for hc in range(16):
                            for t in range(4):
                                mmb(psq[t], xqt[:, hc, t * P:(t + 1) * P],
                                    w[:, hc, :], hc == 0, hc == 15)
                        for t in range(4):
                            nc.vector.tensor_scalar_mul(
                                qa_t[t][:, f * 512:(f + 1) * 512], psq[t],
                                sqq[t][:])
                    for t in range(4):
                        qa = qa_t[t]
                        s2 = p_qa.tile([P, 1], f32, tag="s2", name="s2")
                        rms_scale(p_qa, s2[:], qa[:], QLR, "qa")
                        qab = p_qa.tile([P, QLR], bf16, tag="qab",
                                        name="qab")
                        nc.vector.tensor_scalar_mul(qab[:], qa[:], s2[:])
                        for g in range(3):
                            tr4b(qanT[:, 4 * g:4 * (g + 1),
                                      t * P:(t + 1) * P],
                                 [qab[:, (4 * g + k) * P:(4 * g + k + 1) * P]
                                  for k in range(4)])
                # q_b^T: out [qhd-chunk, tok]; 24 chunks (16 nope + 8 pe)
                wqbr = wqb.rearrange("(c p) f -> p c f", p=P)
                with tc.tile_pool(name="p_qb", bufs=2) as p_qb:
                    for op in range(12):  # pairs of output chunks
                        w = p_qb.tile([P, 12, 256], bf16, tag="wqbt",
                                      name="wqbt")
                        nc.sync.dma_start(
                            w[:], wqbr[:, :, op * 256:(op + 1) * 256])
                        for oh in range(2):
                            oc = 2 * op + oh
                            po = pb()
                            for lc in range(12):
                                mmb(po, w[:, lc, oh * P:(oh + 1) * P],
                                    qanT[:, lc, :], lc == 0, lc == 11)
                            if oc < 16:
                                nc.any.tensor_copy(qT[:, oc, :], po)
                            else:
                                nc.any.tensor_copy(QTpe2[:, oc - 16, :], po)
                # rope q_pe (2 heads per slot; rmat is block-diag 2x64)
                for c in range(8):
                    qd = QTpe2[:, c, :]
                    psw = pb()
                    mmb(psw, rmatb_s[:], qd, True, True)
                    t1 = p_qs.tile([P, TQ], bf16, tag="qrt1", name="qrt1")
                    t2 = p_qs.tile([P, TQ], bf16, tag="qrt2", name="qrt2")
                    nc.vector.tensor_mul(t1[:], qd, cq[:])
                    nc.vector.tensor_mul(t2[:], psw[:], sq_[:])
                    nc.vector.tensor_add(qd, t1[:], t2[:])

            s_xq.close()

            # ========================================================
            # Attention: transposed-softmax layout
            # ========================================================
            p_at = s_att.enter_context(
                tc.tile_pool(name="p_at", bufs=1, side="right"))
            attnT = p_at.tile([P, NH, TQ], bf16, name="attnT")
            with tc.tile_pool(name="p_b1", bufs=1) as p_b1, \
                 tc.tile_pool(name="p_bs", bufs=2) as p_bs:
                wk_s = p_b1.tile([P, 4, NH * NOPE], bf16, name="wk_s")
                nc.sync.dma_start(
                    wk_s[:], wk.rearrange("(c p) f -> p c f", p=P))
                wv_s = p_b1.tile([P, 4, NH * VD], bf16, name="wv_s")
                nc.sync.dma_start(
                    wv_s[:], wv.rearrange("(c p) f -> p c f", p=P))
                masks = p_b1.tile([P, 16, 256], f32, name="masks")
                nc.sync.dma_start(
                    masks[:], maskt.rearrange("p (c f) -> p c f", f=256))
                for h in range(NH):
                    g4, hh, par = h // 4, h % 4, h % 2
                    # K^T for this head: [nope, keys]
                    kt = p_b1.tile([P, S], bf16, tag="kt", name="kt",
                                   bufs=2)
                    for g in range(4):
                        pk = pb()
                        for cc in range(4):
                            mmb(pk, wk_s[:, cc, h * NOPE:(h + 1) * NOPE],
                                ckvt[:, cc, g * 512:(g + 1) * 512],
                                cc == 0, cc == 3)
                        nc.any.tensor_copy(kt[:, g * 512:(g + 1) * 512], pk)
                    # V for 4-head group: [keys, vd(4 heads)]
                    if hh == 0:
                        v4 = p_b1.tile([P, 16, 512], bf16, tag="v4",
                                       name="v4", bufs=2)
                        for kc in range(16):
                            pv = pb()
                            for cc in range(4):
                                mmb(pv,
                                    ckvt[:, cc, kc * P:(kc + 1) * P],
                                    wv_s[:, cc, g4 * 512:(g4 + 1) * 512],
                                    cc == 0, cc == 3)
                            nc.any.tensor_copy(v4[:, kc, :], pv)
                    # pass 1: scores^T -> mask -> exp -> probs (bf16)
                    probs = p_b1.tile([P, 16, TQ], bf16, tag="probs",
                                      name="probs", bufs=2)
                    for kc in range(16):
                        fq = FQ[kc] * P
                        s = ppA.tile([P, TQ], f32, tag="a", name="sc")
                        mmb(s[:, fq:], kt[:, kc * P:(kc + 1) * P],
                            qT[:, h, fq:], True, False)
                        nc.tensor.matmul(
                            s[:, fq:],
                            ckvt[64 * par:64 * par + 64, 4,
                                 kc * P:(kc + 1) * P],
                            QTpe2[64 * par:64 * par + 64, h // 2, fq:],
                            start=False, stop=True)
                        mo = FQ[kc] * P
                        mwd = MW[kc] * P
                        nc.vector.tensor_add(
                            s[:, mo:mo + mwd], s[:, mo:mo + mwd],
                            masks[:, kc, 0:mwd])
                        nc.scalar.activation(probs[:, kc, fq:], s[:, fq:],
                                             AF.Exp, scale=SCALE)
                    # pass 2: AV + denominator accumulate over kc
                    pav = pt_()
                    pdn = pt_()
                    for kc in range(16):
                        fq = FQ[kc] * P
                        mmb(pav[:, fq:],
                            v4[:, kc, hh * P:(hh + 1) * P],
                            probs[:, kc, fq:], kc == 0, kc == 15, skip=True)
                        mmb(pdn[0:1, fq:], onesb[:],
                            probs[:, kc, fq:], kc == 0, kc == 15, skip=True)
                    dr = p_bs.tile([1, TQ], f32, tag="dr", name="dr")
                    nc.vector.reciprocal(dr[:], pdn[0:1, :])
                    dbc = p_bs.tile([P, TQ], f32, tag="dbc", name="dbc")
                    nc.gpsimd.partition_broadcast(dbc[:], dr[0:1, :])
                    nc.vector.tensor_mul(attnT[:, h, :], pav[:], dbc[:])
            s_kvb.close()

        # ============================================================
        # C: o-proj + residual; MLP
        # ============================================================
        p_c = s_c.enter_context(tc.tile_pool(name="p_c", bufs=1))
        acc = [p_c.tile([P, H], f32, tag="acc%d" % t, name="acc%d" % t)
               for t in range(4)]
        with tc.tile_pool(name="p_cs", bufs=2) as p_cs:
            wor = wo.rearrange("(c p) f -> p c f", p=P)
            with tc.tile_pool(name="p_co", bufs=2) as p_co:
                for f in range(4):
                    pso = pa(4)
                    w = p_co.tile([P, 16, 512], bf16, tag="wot", name="wot")
                    nc.sync.dma_start(
                        w[:], wor[:, :, f * 512:(f + 1) * 512])
                    for hc in range(16):
                        for t in range(4):
                            mmb(pso[t], attnT[:, hc, t * P:(t + 1) * P],
                                w[:, hc, :], hc == 0, hc == 15)
                    for t in range(4):
                        res = p_co.tile([P, 512], f32, tag="res", name="res")
                        nc.sync.dma_start(
                            res[:], xq[t * P:(t + 1) * P,
                                       f * 512:(f + 1) * 512])
                        nc.vector.tensor_add(
                            acc[t][:, f * 512:(f + 1) * 512],
                            pso[t], res[:])
            s_att.close()
            s_qtp.close()

            # y = rms(h1) -> yT
            yT = p_c.tile([P, 16, TQ], f32, name="yT")
            for t in range(4):
                s3 = p_cs.tile([P, 1], f32, tag="s3", name="s3")
                rms_scale(p_cs, s3[:], acc[t][:], H, "y2ksq")
                yn = p_cs.tile([P, H], f32, tag="y2ksqsq", name="yn")
                nc.vector.tensor_scalar_mul(yn[:], acc[t][:], s3[:])
                for g in range(4):
                    tr4(yT[:, 4 * g:4 * (g + 1), t * P:(t + 1) * P],
                        [yn[:, (4 * g + k) * P:(4 * g + k + 1) * P]
                         for k in range(4)])

            # MLP
            with tc.tile_pool(name="wbig", bufs=3) as wbig, \
                 tc.tile_pool(name="mtp", bufs=2) as mtp:
                for fs in range(IMP // 512):
                    wgt = wbig.tile([P, 16, 512], f32, tag="wbig",
                                    name="wgt")
                    wut = wbig.tile([P, 16, 512], f32, tag="wbig",
                                    name="wut")
                    for hc in range(16):
                        dmaf(wgt[:, hc, :],
                                          wg[hc * P:(hc + 1) * P,
                                             fs * 512:(fs + 1) * 512])
                        dmaf(wut[:, hc, :],
                                          wu[hc * P:(hc + 1) * P,
                                             fs * 512:(fs + 1) * 512])
                    wdt = wbig.tile([P, 4, H], f32, tag="wbig", name="wdt")
                    for ic in range(4):
                        dmaf(wdt[:, ic, :],
                                          wd[(fs * 4 + ic) * P:
                                             (fs * 4 + ic + 1) * P, :])
                    mt = mtp.tile([P, 4, TQ], f32, tag="mt", name="mt",
                                  bufs=1)
                    for t in range(4):
                        psg, psu = pb(2)
                        for hc in range(16):
                            mm(psg, yT[:, hc, t * P:(t + 1) * P],
                               wgt[:, hc, :], hc == 0, hc == 15)
                            mm(psu, yT[:, hc, t * P:(t + 1) * P],
                               wut[:, hc, :], hc == 0, hc == 15)
                        gs = p_cs.tile([P, 512], f32, tag="gs", name="gs")
                        nc.scalar.activation(gs[:], psg, AF.Silu)
                        mn = p_cs.tile([P, 512], f32, tag="mn", name="mn")
                        nc.vector.tensor_mul(mn[:], gs[:], psu)
                        tr4(mt[:, :, t * P:(t + 1) * P],
                            [mn[:, k * P:(k + 1) * P] for k in range(4)])
                    for t in range(4):
                        psd = pa(4)
                        for ic in range(4):
                            for f in range(4):
                                mm(psd[f], mt[:, ic, t * P:(t + 1) * P],
                                   wdt[:, ic, f * 512:(f + 1) * 512],
                                   ic == 0, ic == 3)
                        for f in range(4):
                            nc.vector.tensor_add(
                                acc[t][:, f * 512:(f + 1) * 512],
                                acc[t][:, f * 512:(f + 1) * 512], psd[f])

            for t in range(4):
                nc.sync.dma_start(out[t * P:(t + 1) * P, :], acc[t][:])
        s_c.close()

    nc.compile()
    return nc


# ---------------------------------------------------------------------------
# host side
# ---------------------------------------------------------------------------

_DEINT = np.concatenate([np.arange(0, ROPE, 2), np.arange(1, ROPE, 2)])
BF = ml_dtypes.bfloat16


def _rmat():
    r = np.zeros((P, P), np.float32)
    for m in range(P):
        base = (m // 64) * 64
        k = base + ((m - base) + 32) % 64
        r[k, m] = 1.0
    return r


def _prep_core(c, hs, ins):
    b, j = c // 4, c % 4
    qch = [j, 4 + j, 8 + j, 12 + j]
    qrows = np.concatenate([np.arange(ch * P, (ch + 1) * P) for ch in qch])

    pos = np.asarray(ins["position_ids"])[b].astype(np.int64)
    cosg = np.asarray(ins["cos"])[pos]           # [S, 64]
    sing = np.asarray(ins["sin"])[pos]
    ssgn = np.concatenate([-sing[:, :32], sing[:, 32:]], 1)

    def dup(x):  # [S,64] -> [128, S]
        xt = np.ascontiguousarray(x.T.astype(np.float32))
        return np.concatenate([xt, xt], 0)

    iln = np.asarray(ins["input_ln_w"])[:, None]
    qln = np.asarray(ins["q_a_ln_w"])[:, None]
    kln = np.asarray(ins["kv_a_ln_w"])[:, None]
    pln = np.asarray(ins["post_ln_w"])[:, None]

    wqa = (iln * np.asarray(ins["q_a_kernel"])).astype(np.float32)
    wqb_ = (qln * np.asarray(ins["q_b_kernel"])).reshape(QLR, NH, QHD)
    wqb = np.concatenate(
        [wqb_[:, :, :NOPE].reshape(QLR, NH * NOPE),
         wqb_[:, :, NOPE:][:, :, _DEINT].reshape(QLR, NH * ROPE)], 1)
    kva = iln * np.asarray(ins["kv_a_kernel"])
    rope_d = kva[:, KVLR:][:, _DEINT]
    wkva = np.concatenate([kva[:, :KVLR], rope_d, rope_d], 1)
    wkb = (kln * np.asarray(ins["kv_b_kernel"])).reshape(KVLR, NH, NOPE + VD)
    wk = wkb[:, :, :NOPE].reshape(KVLR, NH * NOPE)
    wv = wkb[:, :, NOPE:].reshape(KVLR, NH * VD)
    wg = np.zeros((H, IMP), np.float32)
    wg[:, :IM] = pln * np.asarray(ins["gate_kernel"])
    wu = np.zeros((H, IMP), np.float32)
    wu[:, :IM] = pln * np.asarray(ins["up_kernel"])
    wd = np.zeros((IMP, H), np.float32)
    wd[:IM, :] = np.asarray(ins["down_kernel"])

    # masks: for key chunk kc, q chunks FQ[kc]..FQ[kc]+MW[kc]-1 get a
    # 0/NEG additive causal mask [128 keys, 128 q] each
    maskt = np.zeros((P, 16, 256), np.float32)
    pp = np.arange(P)
    for kc in range(16):
        for w in range(MW[kc]):
            qc = FQ[kc] + w  # local q chunk
            gq = (4 * qc + j) * P + pp[None, :]   # global q position
            gk = kc * P + pp[:, None]             # global k position
            maskt[:, kc, w * P:(w + 1) * P] = np.where(
                gq >= gk, np.float32(0), np.float32(NEG))

    f32c = lambda x: np.ascontiguousarray(x, dtype=np.float32)
    bfc = lambda x: np.ascontiguousarray(np.asarray(x, np.float32),
                                         dtype=BF)
    return {
        "hidb": bfc(hs[b]),
        "xqb": bfc(hs[b][qrows]),
        "xq": f32c(hs[b][qrows]),
        "wqa": bfc(wqa),
        "wqb": bfc(wqb),
        "wkva": bfc(wkva),
        "wk": bfc(wk),
        "wv": bfc(wv),
        "wo": bfc(np.asarray(ins["o_kernel"])),
        "wg": f32c(wg),
        "wu": f32c(wu),
        "wd": f32c(wd),
        "cosq": bfc(dup(cosg)[:, qrows]),
        "sinq": bfc(dup(ssgn)[:, qrows]),
        "cosk": f32c(dup(cosg)),
        "sink": f32c(dup(ssgn)),
        "maskt": maskt.reshape(P, 16 * 256),
        "rmat": _rmat(),
        "rmatb": bfc(_rmat()),
    }, qrows


def kernel(**inputs):
    from concourse import bass_utils

    hs = np.asarray(inputs["hidden_states"], dtype=np.float32)
    in_maps, qrows_l = [], []
    for c in range(8):
        m, qr = _prep_core(c, hs, inputs)
        in_maps.append(m)
        qrows_l.append(qr)

    nc = _build()
    res = bass_utils.run_bass_kernel_spmd(
        nc, in_maps, core_ids=list(range(8)))

    out = np.empty((B, S, H), np.float32)
    for c in range(8):
        out[c // 4, qrows_l[c]] = res.results[c]["out"]
    return out
